# revision 4
# baseline (speedup 1.0000x reference)
"""TRN2 Bass kernel for nn_Attention_17935783428543.

Reference computation (per batch b of 4):
  qkv = w_qkv @ X        (X = x[b] as [C=128, N=4096])
  per head h (4 heads, d=32): sim = (q_h * scale)^T k_h ; P = softmax(sim)
  y_h = P @ v_h^T ; out = w_out @ concat_h(y_h^T) + b_out

Sharding: 8 cores = 4 batches x 2 query-halves. Each core computes the full
attention for its batch restricted to 2048 query pixels (all 4096 keys), all
4 heads, including QKV projection and the output projection. No collectives.

Per-core layout decisions (all matmuls bf16, PSUM f32):
  - sim^T tiles [j=128, i=512] per head; 4 heads packed in the 128x128 PE
    via tile_position row-packing (K=32 each).
  - exp on ScalarE in one instruction over a 4-bank PSUM tensor [128, 2048],
    scale folds the 1/sqrt(d) factor; output bf16 P^T in SBUF.
  - P@V via col-packed matmuls (M=32 per head) accumulating y^T [4h*32d, i]
    in one PSUM bank; softmax denominators via ones[128,32] matmuls that
    broadcast each head's row-sum to its 32-row block (M is free on PE).
  - normalize + w_out^T projection + bias per i-chunk epilogue.

The query-half assignment uses a host-side rotation of x's pixel axis so all
8 cores run the identical SPMD graph: queries are always columns 0:2048.
"""

import numpy as np
import ml_dtypes

import concourse.mybir as mybir
import concourse.tile as tile
from concourse import bacc
from concourse.bass_utils import run_bass_kernel_spmd

F32 = mybir.dt.float32
BF16 = mybir.dt.bfloat16
NPBF16 = ml_dtypes.bfloat16

B = 4
C = 128
HEADS = 4
D = 32
N = 4096          # pixels per batch (64*64)
NQ = 2048         # query pixels per core
SCALE = D ** -0.5
I_CHUNK = 512
J_CHUNK = 128
N_I = NQ // I_CHUNK     # 4
N_J = N // J_CHUNK      # 32

_NC_CACHE = {}


def _build_nc():
    nc = bacc.Bacc("TRN2", target_bir_lowering=False, debug=False, num_devices=8)

    x = nc.dram_tensor("x", [C, N], BF16, kind="ExternalInput").ap()
    wq = nc.dram_tensor("wq", [C, C], BF16, kind="ExternalInput").ap()
    wk = nc.dram_tensor("wk", [C, C], BF16, kind="ExternalInput").ap()
    wv = nc.dram_tensor("wv", [C, C], BF16, kind="ExternalInput").ap()
    wo = nc.dram_tensor("wo", [C, C], BF16, kind="ExternalInput").ap()
    bo = nc.dram_tensor("bo", [C, 1], F32, kind="ExternalInput").ap()
    out = nc.dram_tensor("out", [C, NQ], F32, kind="ExternalOutput").ap()

    with tile.TileContext(nc) as tc:
        with (
            tc.tile_pool(name="const", bufs=1) as cpool,
            tc.tile_pool(name="acts", bufs=1) as apool,
            tc.tile_pool(name="pt", bufs=3) as ptpool,
            tc.tile_pool(name="epi", bufs=2) as epool,
            tc.tile_pool(name="psum_proj", bufs=2, space="PSUM") as pproj,
            tc.tile_pool(name="psum_sim", bufs=1, space="PSUM") as psim,
            tc.tile_pool(name="psum_acc", bufs=1, space="PSUM") as pacc,
        ):
            # ---- constants / weights ----
            wq_sb = cpool.tile([C, C], BF16, tag="wq")
            nc.sync.dma_start(wq_sb[:], wq)
            wk_sb = cpool.tile([C, C], BF16, tag="wk")
            nc.sync.dma_start(wk_sb[:], wk)
            wv_sb = cpool.tile([C, C], BF16, tag="wv")
            nc.sync.dma_start(wv_sb[:], wv)
            wo_sb = cpool.tile([C, C], BF16, tag="wo")
            nc.sync.dma_start(wo_sb[:], wo)
            bo_sb = cpool.tile([C, 1], F32, tag="bo")
            nc.sync.dma_start(bo_sb[:], bo)
            ones32 = cpool.tile([128, 32], BF16, tag="ones32")
            nc.vector.memset(ones32[:], 1.0)

            # warm the ACT exp table during the DMA prologue
            warm = cpool.tile([1, 1], F32, tag="warm")
            nc.vector.memset(warm[:], 0.0)
            nc.scalar.activation(warm[:], warm[:], mybir.ActivationFunctionType.Exp)

            # ---- x and projections ----
            x_sb = apool.tile([C, N], BF16, tag="x")
            for g in range(N // 512):
                nc.sync.dma_start(x_sb[:, 512 * g : 512 * (g + 1)], x[:, 512 * g : 512 * (g + 1)])

            q_all = apool.tile([C, NQ], BF16, tag="q")    # [4h*32c', i]
            k_all = apool.tile([C, N], BF16, tag="k")     # [4h*32c', j]
            vT_all = apool.tile([C, N], BF16, tag="vT")   # chunk J cols J*128.. : [j, 4h*32d]

            # q projection (queries are x cols 0:2048)
            for g in range(NQ // 512):
                sl = slice(512 * g, 512 * (g + 1))
                ps = pproj.tile([128, 512], F32, tag="proj")
                nc.tensor.matmul(ps[:], lhsT=wq_sb[:], rhs=x_sb[:, sl], start=True, stop=True)
                nc.vector.tensor_copy(q_all[:, sl], ps[:])
            # k projection (all pixels)
            for g in range(N // 512):
                sl = slice(512 * g, 512 * (g + 1))
                ps = pproj.tile([128, 512], F32, tag="proj")
                nc.tensor.matmul(ps[:], lhsT=wk_sb[:], rhs=x_sb[:, sl], start=True, stop=True)
                nc.vector.tensor_copy(k_all[:, sl], ps[:])
            # vT projection: out[j, o'] = sum_c x[c, j] wv[c, o'], per 128-pixel chunk
            for g in range(N // 512):
                ps = pproj.tile([128, 512], F32, tag="proj")
                for c4 in range(4):
                    J = 4 * g + c4
                    nc.tensor.matmul(
                        ps[:, 128 * c4 : 128 * (c4 + 1)],
                        lhsT=x_sb[:, 128 * J : 128 * (J + 1)],
                        rhs=wv_sb[:],
                        start=True,
                        stop=True,
                    )
                nc.vector.tensor_copy(vT_all[:, 512 * g : 512 * (g + 1)], ps[:])

            # ---- attention main loop (software pipelined) ----
            # body(t): sims(t) -> exp(t) -> PV/sums(t-1) -> deferred epilogue.
            # PV/sums of the previous round run on the PE while ACT streams
            # exp(t); the i-chunk epilogue is deferred one extra round so its
            # DVE chain (recip etc.) completes before PV of the next i-chunk
            # needs the yT/sums PSUM banks.
            acc_tiles = {}

            def emit_pv_sums(I, J, pT):
                for h in range(HEADS):
                    nc.tensor.matmul(
                        acc_tiles[I][0][32 * h : 32 * h + 32, :],
                        lhsT=vT_all[:, 128 * J + 32 * h : 128 * J + 32 * h + 32],
                        rhs=pT[:, I_CHUNK * h : I_CHUNK * (h + 1)],
                        start=(J == 0),
                        stop=(J == N_J - 1),
                        tile_position=(0, 32 * h),
                    )
                for h in range(HEADS):
                    nc.tensor.matmul(
                        acc_tiles[I][1][32 * h : 32 * h + 32, :],
                        lhsT=ones32[:],
                        rhs=pT[:, I_CHUNK * h : I_CHUNK * (h + 1)],
                        start=(J == 0),
                        stop=(J == N_J - 1),
                        tile_position=(0, 32 * h),
                    )

            def emit_epilogue(I):
                yT, sums = acc_tiles.pop(I)
                isl = slice(I_CHUNK * I, I_CHUNK * (I + 1))
                s_r = epool.tile([128, I_CHUNK], F32, tag="s_r")
                nc.vector.reciprocal_approx_fast(s_r[:], sums[:])
                ynorm = epool.tile([128, I_CHUNK], BF16, tag="ynorm")
                nc.vector.tensor_tensor(ynorm[:], yT[:], s_r[:], mybir.AluOpType.mult)
                op = pproj.tile([128, I_CHUNK], F32, tag="proj")
                nc.tensor.matmul(op[:], lhsT=wo_sb[:], rhs=ynorm[:], start=True, stop=True)
                out_sb = epool.tile([128, I_CHUNK], F32, tag="out_sb")
                nc.vector.tensor_scalar_add(out_sb[:], op[:], bo_sb[:, :])
                nc.sync.dma_start(out[:, isl], out_sb[:])

            rounds = [(I, J) for I in range(N_I) for J in range(N_J)]
            pending_pv = None
            for I, J in rounds:
                if J == 0:
                    acc_tiles[I] = (
                        pacc.tile([128, I_CHUNK], F32, tag="yT", name="yT"),
                        pacc.tile([128, I_CHUNK], F32, tag="sums", name="sums"),
                    )
                simT = psim.tile([128, 4 * I_CHUNK], F32, tag="simT")
                for h in range(HEADS):
                    nc.tensor.matmul(
                        simT[:, I_CHUNK * h : I_CHUNK * (h + 1)],
                        lhsT=k_all[32 * h : 32 * h + 32, 128 * J : 128 * (J + 1)],
                        rhs=q_all[32 * h : 32 * h + 32, I_CHUNK * I : I_CHUNK * (I + 1)],
                        start=True,
                        stop=True,
                        tile_position=(32 * h, 0),
                    )
                pT = ptpool.tile([128, 4 * I_CHUNK], BF16, tag="pT")
                # exp split 1536+512: frees heads 0-2's sim banks while head 3's
                # exp still runs, so next round's sims overlap the exps and ACT
                # never waits on the sim WAR (1536 is also an ACT sweet spot).
                nc.scalar.activation(
                    pT[:, : 3 * I_CHUNK],
                    simT[:, : 3 * I_CHUNK],
                    mybir.ActivationFunctionType.Exp,
                    scale=SCALE,
                )
                nc.scalar.activation(
                    pT[:, 3 * I_CHUNK :],
                    simT[:, 3 * I_CHUNK :],
                    mybir.ActivationFunctionType.Exp,
                    scale=SCALE,
                )
                if pending_pv is not None:
                    pI, pJ, ppT = pending_pv
                    emit_pv_sums(pI, pJ, ppT)
                    if pJ == N_J - 1:
                        emit_epilogue(pI)
                pending_pv = (I, J, pT)
            pI, pJ, ppT = pending_pv
            emit_pv_sums(pI, pJ, ppT)
            emit_epilogue(pI)

    nc.compile()
    return nc


def kernel(x, w_qkv, w_out, b_out, _trace=False):
    if "nc" not in _NC_CACHE:
        _NC_CACHE["nc"] = _build_nc()
    nc = _NC_CACHE["nc"]

    x = np.asarray(x, dtype=np.float32).reshape(B, C, N)
    w_qkv = np.asarray(w_qkv, dtype=np.float32)
    w_out = np.asarray(w_out, dtype=np.float32)
    b_out = np.asarray(b_out, dtype=np.float32)

    wq = np.ascontiguousarray(w_qkv[0:C].T).astype(NPBF16)
    wk = np.ascontiguousarray(w_qkv[C : 2 * C].T).astype(NPBF16)
    wv = np.ascontiguousarray(w_qkv[2 * C : 3 * C].T).astype(NPBF16)
    wo = np.ascontiguousarray(w_out.T).astype(NPBF16)
    bo = np.ascontiguousarray(b_out.reshape(C, 1))

    in_maps = []
    for core in range(8):
        b, half = core >> 1, core & 1
        xb = x[b]
        if half:
            xb = np.concatenate([xb[:, NQ:], xb[:, :NQ]], axis=1)
        in_maps.append(
            {
                "x": np.ascontiguousarray(xb).astype(NPBF16),
                "wq": wq,
                "wk": wk,
                "wv": wv,
                "wo": wo,
                "bo": bo,
            }
        )

    res = run_bass_kernel_spmd(nc, in_maps, list(range(8)), trace=_trace)

    full = np.empty((B, C, N), np.float32)
    for core in range(8):
        b, half = core >> 1, core & 1
        full[b][:, NQ * half : NQ * (half + 1)] = res.results[core]["out"]
    out = full.reshape(B, C, 64, 64)
    if _trace:
        return out, res
    return out


# revision 6
# speedup vs baseline: 1.6226x; 1.6226x over previous
"""TRN2 Bass kernel for nn_Attention_17935783428543.

Reference computation (per batch b of 4):
  qkv = w_qkv @ X        (X = x[b] as [C=128, N=4096])
  per head h (4 heads, d=32): sim = (q_h * scale)^T k_h ; P = softmax(sim)
  y_h = P @ v_h^T ; out = w_out @ concat_h(y_h^T) + b_out

Sharding: 8 cores = 4 batches x 2 query-halves. Each core computes the full
attention for its batch restricted to 2048 query pixels (all 4096 keys), all
4 heads, including QKV projection and the output projection. No collectives.

Per-core layout decisions (all matmuls bf16, PSUM f32):
  - sim^T tiles [j=128, i=512] per head; 4 heads packed in the 128x128 PE
    via tile_position row-packing (K=32 each).
  - exp on ScalarE in one instruction over a 4-bank PSUM tensor [128, 2048],
    scale folds the 1/sqrt(d) factor; output bf16 P^T in SBUF.
  - P@V via col-packed matmuls (M=32 per head) accumulating y^T [4h*32d, i]
    in one PSUM bank; softmax denominators via ones[128,32] matmuls that
    broadcast each head's row-sum to its 32-row block (M is free on PE).
  - normalize + w_out^T projection + bias per i-chunk epilogue.

The query-half assignment uses a host-side rotation of x's pixel axis so all
8 cores run the identical SPMD graph: queries are always columns 0:2048.
"""

import numpy as np
import ml_dtypes

import concourse.mybir as mybir
import concourse.tile as tile
from concourse import bacc
from concourse.bass_utils import run_bass_kernel_spmd

F32 = mybir.dt.float32
BF16 = mybir.dt.bfloat16
NPBF16 = ml_dtypes.bfloat16

B = 4
C = 128
HEADS = 4
D = 32
N = 4096          # pixels per batch (64*64)
NQ = 2048         # query pixels per core
SCALE = D ** -0.5
I_CHUNK = 512
J_CHUNK = 128
N_I = NQ // I_CHUNK     # 4
N_J = N // J_CHUNK      # 32

_NC_CACHE = {}


def _build_nc():
    nc = bacc.Bacc("TRN2", target_bir_lowering=False, debug=False, num_devices=8)

    x = nc.dram_tensor("x", [C, N], BF16, kind="ExternalInput").ap()
    wq = nc.dram_tensor("wq", [C, C], BF16, kind="ExternalInput").ap()
    wk = nc.dram_tensor("wk", [C, C], BF16, kind="ExternalInput").ap()
    wv = nc.dram_tensor("wv", [C, C], BF16, kind="ExternalInput").ap()
    wo = nc.dram_tensor("wo", [C, C], BF16, kind="ExternalInput").ap()
    bo = nc.dram_tensor("bo", [C, 1], F32, kind="ExternalInput").ap()
    out = nc.dram_tensor("out", [C, NQ], F32, kind="ExternalOutput").ap()

    with tile.TileContext(nc) as tc:
        with (
            tc.tile_pool(name="const", bufs=1) as cpool,
            tc.tile_pool(name="acts", bufs=1) as apool,
            tc.tile_pool(name="pt", bufs=3) as ptpool,
            tc.tile_pool(name="epi", bufs=2) as epool,
            tc.tile_pool(name="psum_proj", bufs=2, space="PSUM") as pproj,
            tc.tile_pool(name="psum_sim", bufs=1, space="PSUM") as psim,
            tc.tile_pool(name="psum_acc", bufs=1, space="PSUM") as pacc,
        ):
            # ---- constants / weights ----
            wq_sb = cpool.tile([C, C], BF16, tag="wq")
            nc.sync.dma_start(wq_sb[:], wq)
            wk_sb = cpool.tile([C, C], BF16, tag="wk")
            nc.sync.dma_start(wk_sb[:], wk)
            wv_sb = cpool.tile([C, C], BF16, tag="wv")
            nc.sync.dma_start(wv_sb[:], wv)
            wo_sb = cpool.tile([C, C], BF16, tag="wo")
            nc.sync.dma_start(wo_sb[:], wo)
            bo_sb = cpool.tile([C, 1], F32, tag="bo")
            nc.sync.dma_start(bo_sb[:], bo)
            ones32 = cpool.tile([128, 32], BF16, tag="ones32")
            nc.vector.memset(ones32[:], 1.0)

            # warm the ACT exp table during the DMA prologue
            warm = cpool.tile([1, 1], F32, tag="warm")
            nc.vector.memset(warm[:], 0.0)
            nc.scalar.activation(warm[:], warm[:], mybir.ActivationFunctionType.Exp)

            # ---- x and projections ----
            x_sb = apool.tile([C, N], BF16, tag="x")
            for g in range(N // 512):
                nc.sync.dma_start(x_sb[:, 512 * g : 512 * (g + 1)], x[:, 512 * g : 512 * (g + 1)])

            q_all = apool.tile([C, NQ], BF16, tag="q")    # [4h*32c', i]
            k_all = apool.tile([C, N], BF16, tag="k")     # [4h*32c', j]
            vT_all = apool.tile([C, N], BF16, tag="vT")   # chunk J cols J*128.. : [j, 4h*32d]

            # q projection (queries are x cols 0:2048)
            for g in range(NQ // 512):
                sl = slice(512 * g, 512 * (g + 1))
                ps = pproj.tile([128, 512], F32, tag="proj")
                nc.tensor.matmul(ps[:], lhsT=wq_sb[:], rhs=x_sb[:, sl], start=True, stop=True)
                nc.vector.tensor_copy(q_all[:, sl], ps[:])
            # k projection (all pixels)
            for g in range(N // 512):
                sl = slice(512 * g, 512 * (g + 1))
                ps = pproj.tile([128, 512], F32, tag="proj")
                nc.tensor.matmul(ps[:], lhsT=wk_sb[:], rhs=x_sb[:, sl], start=True, stop=True)
                nc.vector.tensor_copy(k_all[:, sl], ps[:])
            # vT projection: out[j, o'] = sum_c x[c, j] wv[c, o'], per 128-pixel chunk
            for g in range(N // 512):
                ps = pproj.tile([128, 512], F32, tag="proj")
                for c4 in range(4):
                    J = 4 * g + c4
                    nc.tensor.matmul(
                        ps[:, 128 * c4 : 128 * (c4 + 1)],
                        lhsT=x_sb[:, 128 * J : 128 * (J + 1)],
                        rhs=wv_sb[:],
                        start=True,
                        stop=True,
                    )
                nc.vector.tensor_copy(vT_all[:, 512 * g : 512 * (g + 1)], ps[:])

            # ---- attention main loop (software pipelined) ----
            # body(t): sims(t) -> exp(t) -> PV/sums(t-1) -> deferred epilogue.
            # PV/sums of the previous round run on the PE while ACT streams
            # exp(t); the i-chunk epilogue is deferred one extra round so its
            # DVE chain (recip etc.) completes before PV of the next i-chunk
            # needs the yT/sums PSUM banks.
            acc_tiles = {}

            def emit_pv_sums(I, J, pT_a, pT_b):
                def p_slice(h):
                    return pT_a[:, I_CHUNK * h : I_CHUNK * (h + 1)] if h < 3 else pT_b[:]

                for h in range(HEADS):
                    nc.tensor.matmul(
                        acc_tiles[I][0][32 * h : 32 * h + 32, :],
                        lhsT=vT_all[:, 128 * J + 32 * h : 128 * J + 32 * h + 32],
                        rhs=p_slice(h),
                        start=(J == 0),
                        stop=(J == N_J - 1),
                        tile_position=(0, 32 * h),
                    )
                for h in range(HEADS):
                    nc.tensor.matmul(
                        acc_tiles[I][1][32 * h : 32 * h + 32, :],
                        lhsT=ones32[:],
                        rhs=p_slice(h),
                        start=(J == 0),
                        stop=(J == N_J - 1),
                        tile_position=(0, 32 * h),
                    )

            def emit_epilogue(I):
                yT, sums = acc_tiles.pop(I)
                isl = slice(I_CHUNK * I, I_CHUNK * (I + 1))
                s_r = epool.tile([128, I_CHUNK], F32, tag="s_r")
                nc.vector.reciprocal_approx_fast(s_r[:], sums[:])
                ynorm = epool.tile([128, I_CHUNK], BF16, tag="ynorm")
                nc.vector.tensor_tensor(ynorm[:], yT[:], s_r[:], mybir.AluOpType.mult)
                op = pproj.tile([128, I_CHUNK], F32, tag="proj")
                nc.tensor.matmul(op[:], lhsT=wo_sb[:], rhs=ynorm[:], start=True, stop=True)
                out_sb = epool.tile([128, I_CHUNK], F32, tag="out_sb")
                nc.vector.tensor_scalar_add(out_sb[:], op[:], bo_sb[:, :])
                nc.sync.dma_start(out[:, isl], out_sb[:])

            rounds = [(I, J) for I in range(N_I) for J in range(N_J)]
            pending_pv = None
            for I, J in rounds:
                if J == 0:
                    acc_tiles[I] = (
                        pacc.tile([128, I_CHUNK], F32, tag="yT", name="yT"),
                        pacc.tile([128, I_CHUNK], F32, tag="sums", name="sums"),
                    )
                # exp split 1536+512 across SEPARATE tiles: heads 0-2's sim
                # banks free while head 3's exp still runs, so next round's
                # sims overlap the exps and ACT never waits on the sim WAR
                # (1536 is also an ACT sweet spot). Separate tiles keep Tile's
                # WAR tracking per-group.
                simT_a = psim.tile([128, 3 * I_CHUNK], F32, tag="simT_a", name="simT_a")
                simT_b = psim.tile([128, I_CHUNK], F32, tag="simT_b", name="simT_b")
                for h in range(HEADS):
                    dst = (
                        simT_a[:, I_CHUNK * h : I_CHUNK * (h + 1)]
                        if h < 3
                        else simT_b[:]
                    )
                    nc.tensor.matmul(
                        dst,
                        lhsT=k_all[32 * h : 32 * h + 32, 128 * J : 128 * (J + 1)],
                        rhs=q_all[32 * h : 32 * h + 32, I_CHUNK * I : I_CHUNK * (I + 1)],
                        start=True,
                        stop=True,
                        tile_position=(32 * h, 0),
                    )
                pT_a = ptpool.tile([128, 3 * I_CHUNK], BF16, tag="pT_a", name="pT_a")
                pT_b = ptpool.tile([128, I_CHUNK], BF16, tag="pT_b", name="pT_b")
                nc.scalar.activation(
                    pT_a[:], simT_a[:], mybir.ActivationFunctionType.Exp, scale=SCALE
                )
                nc.scalar.activation(
                    pT_b[:], simT_b[:], mybir.ActivationFunctionType.Exp, scale=SCALE
                )
                if pending_pv is not None:
                    pI, pJ, pa, pb = pending_pv
                    emit_pv_sums(pI, pJ, pa, pb)
                    if pJ == N_J - 1:
                        emit_epilogue(pI)
                pending_pv = (I, J, pT_a, pT_b)
            pI, pJ, pa, pb = pending_pv
            emit_pv_sums(pI, pJ, pa, pb)
            emit_epilogue(pI)

    nc.compile()
    return nc


def kernel(x, w_qkv, w_out, b_out, _trace=False):
    if "nc" not in _NC_CACHE:
        _NC_CACHE["nc"] = _build_nc()
    nc = _NC_CACHE["nc"]

    x = np.asarray(x, dtype=np.float32).reshape(B, C, N)
    w_qkv = np.asarray(w_qkv, dtype=np.float32)
    w_out = np.asarray(w_out, dtype=np.float32)
    b_out = np.asarray(b_out, dtype=np.float32)

    wq = np.ascontiguousarray(w_qkv[0:C].T).astype(NPBF16)
    wk = np.ascontiguousarray(w_qkv[C : 2 * C].T).astype(NPBF16)
    wv = np.ascontiguousarray(w_qkv[2 * C : 3 * C].T).astype(NPBF16)
    wo = np.ascontiguousarray(w_out.T).astype(NPBF16)
    bo = np.ascontiguousarray(b_out.reshape(C, 1))

    in_maps = []
    for core in range(8):
        b, half = core >> 1, core & 1
        xb = x[b]
        if half:
            xb = np.concatenate([xb[:, NQ:], xb[:, :NQ]], axis=1)
        in_maps.append(
            {
                "x": np.ascontiguousarray(xb).astype(NPBF16),
                "wq": wq,
                "wk": wk,
                "wv": wv,
                "wo": wo,
                "bo": bo,
            }
        )

    res = run_bass_kernel_spmd(nc, in_maps, list(range(8)), trace=_trace)

    full = np.empty((B, C, N), np.float32)
    for core in range(8):
        b, half = core >> 1, core & 1
        full[b][:, NQ * half : NQ * (half + 1)] = res.results[core]["out"]
    out = full.reshape(B, C, 64, 64)
    if _trace:
        return out, res
    return out
